# revision 31
# baseline (speedup 1.0000x reference)
"""1-D nearest-neighbor retrieval kernel for Trainium2 (8 NeuronCores).

For each query x[b], finds argmin_n |input_tensor[n] - x[b]| and returns
accuracy_tensor[argmin].  Queries are sharded across the 8 cores (512 each,
4 query tiles of 128 partitions); the ref/accuracy tables are replicated.

Per-core pipeline (queries in SBUF partitions, refs in the free dim):
  Phase 1 -- segment minima (the O(B*N) bulk):
    - Each chunk of refs is partition-broadcast to [128, F] SBUF by DMA.
    - ScalarE computes dist = |ref - x_p| via activation(Abs, bias=-x_p)
      (bit-identical to the fp32 reference: one subtract + abs).
    - VectorE min-reduces each 128-wide segment: seg[p, s].  The DVE runs
      at 1 elem/cycle for reductions, which is the kernel's floor.
  Phase 2 -- exact argmin from segment minima (per query tile):
    - global min m = reduce_min(seg); first segment with seg == m via
      max_index (first-occurrence semantics match argmin's tie-break);
      concurrent indirect-DMA gathers of that segment's refs and accuracy
      rows; recompute ref - x (bit-identical) and search +-m with
      max_index; pick accuracy[w] via an iota==w one-hot dot product.

A per-chunk ScalarE "fence" (Copy of one bcast element) absorbs the
multi-queue DMA waits once per chunk, keeping per-instruction semaphore
waits cheap.  All comparisons are exact fp32, so the result matches the
jax reference bit-for-bit, including argmin tie-breaks.
"""
from contextlib import ExitStack

import numpy as np

import concourse.bass as bass
import concourse.bacc as bacc
import concourse.tile as tile
from concourse import mybir
from concourse._compat import with_exitstack
from concourse.bass_utils import run_bass_kernel_spmd

P = 128
N_CORES = 8
B = 4096
B_CORE = B // N_CORES  # 512
N = 65536
F = 4096               # refs per chunk (first chunk is split for fast start)
CHUNK_PLAN = [(0, 512), (512, 512), (1024, 1024), (2048, 2048)] + [
    (off, F) for off in range(F, N, F)
]
N_QT = B_CORE // P     # 4 query tiles per core
W = 128                # segment width
S = F // W             # 16 segments per chunk
NSEG = N // W          # 512 segments total
MM = 512               # max moving free dim per matmul

FP32 = mybir.dt.float32
U32 = mybir.dt.uint32
I32 = mybir.dt.int32


@with_exitstack
def _nn_kernel(ctx: ExitStack, tc: tile.TileContext, xq, refs, rseg, aseg, iota, out):
    nc = tc.nc

    bcast_pool = ctx.enter_context(tc.tile_pool(name="bcast", bufs=3))
    dist_pool = ctx.enter_context(tc.tile_pool(name="dist", bufs=4))
    small_pool = ctx.enter_context(tc.tile_pool(name="small", bufs=2))
    persist = ctx.enter_context(tc.tile_pool(name="persist", bufs=1))

    # Issue the first broadcast DMAs before anything else so ScalarE can
    # start as early as possible.
    early = []
    for off, flen in CHUNK_PLAN[:2]:
        bc = bcast_pool.tile([P, F], FP32, tag="bcast", name="bcast")
        nc.sync.dma_start(
            out=bc[:, :flen],
            in_=refs[off : off + flen][None, :].to_broadcast([P, flen]),
        )
        early.append(bc)

    x_sb = persist.tile([P, N_QT], FP32, tag="x_sb")
    nc.sync.dma_start(out=x_sb[:], in_=xq.rearrange("(q p) -> p q", p=P))
    neg_x = persist.tile([P, N_QT], FP32, tag="neg_x")
    nc.vector.tensor_scalar_mul(neg_x[:], x_sb[:], -1.0)

    # Per-qtile segment minima, filled chunk by chunk.
    segs = [
        persist.tile([P, NSEG], FP32, tag=f"seg{qt}", name=f"seg{qt}")
        for qt in range(N_QT)
    ]

    # ---- Phase 1: segment minima ----
    # Each chunk of refs is replicated to all 128 partitions by DMA;
    # ScalarE computes dist = |ref - x_p| (Abs activation with
    # per-partition bias), the DVE runs only the segment min-reduces.
    # A tiny per-chunk ACT "fence" (Copy of one bcast element) absorbs the
    # multi-queue DMA waits once, so the dist ops carry only their cheap
    # embedded WAR wait.
    fdummy = persist.tile([P, 1], FP32, tag="fdummy")
    iota_pw = persist.tile([P, W], FP32, tag="iota_pw")
    nc.sync.dma_start(out=iota_pw[:], in_=iota[None, :].to_broadcast([P, W]))
    stage = persist.tile([P, N_QT], FP32, tag="stage")

    def phase1(off, flen, qt, fence):
        dist = dist_pool.tile([P, F], FP32, tag="dist", name="dist")
        d_call = nc.scalar.activation(
            dist[:, :flen],
            bcast[:, :flen],
            mybir.ActivationFunctionType.Abs,
            bias=neg_x[:, qt : qt + 1],
            scale=1.0,
        )
        bass._add_dep_helper(
            d_call.ins, fence.ins, sync=False, reason="fence before dist"
        )
        nc.vector.tensor_reduce(
            segs[qt][:, off // W : (off + flen) // W],
            dist[:, :flen].rearrange("p (s w) -> p s w", w=W),
            axis=mybir.AxisListType.X,
            op=mybir.AluOpType.min,
        )

    # ---- Phase 2: exact argmin for one query tile ----
    def phase2(qt):
        gmin = small_pool.tile([P, 1], FP32, tag="gmin")
        nc.vector.tensor_reduce(
            gmin[:], segs[qt][:], axis=mybir.AxisListType.X, op=mybir.AluOpType.min
        )
        m8 = small_pool.tile([P, 8], FP32, tag="m8")
        nc.vector.tensor_copy(m8[:], gmin[:, 0:1].to_broadcast([P, 8]))
        s8 = small_pool.tile([P, 8], U32, tag="s8")
        nc.vector.max_index(s8[:], m8[:], segs[qt][:])
        # Gather the winning segment's refs and accuracies for each lane --
        # two independent gathers keyed by the same row index, so their
        # round-trips overlap.
        gref = small_pool.tile([P, W], FP32, tag="gref")
        nc.gpsimd.indirect_dma_start(
            out=gref[:],
            out_offset=None,
            in_=rseg,
            in_offset=bass.IndirectOffsetOnAxis(ap=s8[:, 0:1], axis=0),
        )
        gacc = small_pool.tile([P, W], FP32, tag="gacc")
        nc.gpsimd.indirect_dma_start(
            out=gacc[:],
            out_offset=None,
            in_=aseg,
            in_offset=bass.IndirectOffsetOnAxis(ap=s8[:, 0:1], axis=0),
        )
        # Recompute ref - x for the gathered segment (bit-identical signed
        # diff) and search it for +gmin / -gmin; the smaller found index is
        # the first position with |diff| == gmin.
        dist_w = small_pool.tile([P, W], FP32, tag="dist_w")
        nc.vector.tensor_scalar(
            dist_w[:],
            gref[:],
            x_sb[:, qt : qt + 1],
            None,
            op0=mybir.AluOpType.subtract,
        )
        mpm = small_pool.tile([P, 8], FP32, tag="mpm")
        nc.vector.tensor_copy(mpm[:, 0:4], gmin[:, 0:1].to_broadcast([P, 4]))
        nc.vector.tensor_scalar(
            mpm[:, 4:8],
            gmin[:, 0:1].to_broadcast([P, 4]),
            -1.0,
            None,
            op0=mybir.AluOpType.mult,
        )
        w8 = small_pool.tile([P, 8], U32, tag="w8")
        nc.vector.max_index(w8[:], mpm[:], dist_w[:])
        # Global index = seg * W + within-segment index (fp32 arithmetic is
        # exact for values < 2^24; a not-found slot becomes 2^32-1 in fp32
        # and loses the min).
        wp_f = small_pool.tile([P, 1], FP32, tag="wp_f")
        nc.vector.tensor_copy(wp_f[:], w8[:, 0:1])
        wm_f = small_pool.tile([P, 1], FP32, tag="wm_f")
        nc.vector.tensor_copy(wm_f[:], w8[:, 4:5])
        w_f = small_pool.tile([P, 1], FP32, tag="w_f")
        nc.vector.tensor_tensor(
            out=w_f[:], in0=wp_f[:], in1=wm_f[:], op=mybir.AluOpType.min
        )
        # accuracy[w]: one-hot select via iota == w, then a sum-reduce.
        sel = small_pool.tile([P, W], FP32, tag="sel")
        nc.vector.tensor_tensor(
            out=sel[:],
            in0=iota_pw[:],
            in1=w_f[:, 0:1].to_broadcast([P, W]),
            op=mybir.AluOpType.is_equal,
        )
        nc.vector.tensor_tensor(
            out=sel[:], in0=sel[:], in1=gacc[:], op=mybir.AluOpType.mult
        )
        nc.vector.tensor_reduce(
            stage[:, qt : qt + 1],
            sel[:],
            axis=mybir.AxisListType.X,
            op=mybir.AluOpType.add,
        )

    for ci, (off, flen) in enumerate(CHUNK_PLAN):
        last = ci == len(CHUNK_PLAN) - 1
        if ci < 2:
            bcast = early[ci]
        else:
            bcast = bcast_pool.tile([P, F], FP32, tag="bcast", name="bcast")
            nc.sync.dma_start(
                out=bcast[:, :flen],
                in_=refs[off : off + flen][None, :].to_broadcast([P, flen]),
            )
        fence = nc.scalar.activation(
            fdummy[:], bcast[:, 0:1], mybir.ActivationFunctionType.Copy
        )
        for qt in range(N_QT):
            phase1(off, flen, qt, fence)
            if last:
                phase2(qt)
    nc.sync.dma_start(out=out.rearrange("(q p) -> p q", p=P), in_=stage[:])


_CACHED_NC = None


def _build():
    global _CACHED_NC
    if _CACHED_NC is not None:
        return _CACHED_NC
    nc = bacc.Bacc("TRN2", target_bir_lowering=False, debug=False)
    xq = nc.dram_tensor("xq", [B_CORE], FP32, kind="ExternalInput").ap()
    refs = nc.dram_tensor("refs", [N], FP32, kind="ExternalInput").ap()
    rseg = nc.dram_tensor("rseg", [NSEG, W], FP32, kind="ExternalInput").ap()
    aseg = nc.dram_tensor("aseg", [NSEG, W], FP32, kind="ExternalInput").ap()
    iota = nc.dram_tensor("iota", [W], FP32, kind="ExternalInput").ap()
    out = nc.dram_tensor("out", [B_CORE], FP32, kind="ExternalOutput").ap()
    with tile.TileContext(nc) as tc:
        _nn_kernel(tc, xq, refs, rseg, aseg, iota, out)
    nc.compile()
    _CACHED_NC = nc
    return nc


def kernel(x, input_tensor, accuracy_tensor):
    x = np.asarray(x, dtype=np.float32)
    refs = np.ascontiguousarray(np.asarray(input_tensor, dtype=np.float32))
    acc = np.ascontiguousarray(np.asarray(accuracy_tensor, dtype=np.float32))

    nc = _build()
    rseg = np.ascontiguousarray(refs.reshape(NSEG, W))
    aseg = np.ascontiguousarray(acc.reshape(NSEG, W))
    iota = np.arange(W, dtype=np.float32)
    in_maps = [
        {
            "xq": np.ascontiguousarray(x[i * B_CORE : (i + 1) * B_CORE]),
            "refs": refs,
            "rseg": rseg,
            "aseg": aseg,
            "iota": iota,
        }
        for i in range(N_CORES)
    ]
    res = run_bass_kernel_spmd(nc, in_maps, core_ids=list(range(N_CORES)))
    return np.concatenate([res.results[i]["out"] for i in range(N_CORES)])


# revision 32
# speedup vs baseline: 1.0061x; 1.0061x over previous
"""1-D nearest-neighbor retrieval kernel for Trainium2 (8 NeuronCores).

For each query x[b], finds argmin_n |input_tensor[n] - x[b]| and returns
accuracy_tensor[argmin].  Queries are sharded across the 8 cores (512 each,
4 query tiles of 128 partitions); the ref/accuracy tables are replicated.

Per-core pipeline (queries in SBUF partitions, refs in the free dim):
  Phase 1 -- segment minima (the O(B*N) bulk):
    - Each chunk of refs is partition-broadcast to [128, F] SBUF by DMA.
    - ScalarE computes dist = |ref - x_p| via activation(Abs, bias=-x_p)
      (bit-identical to the fp32 reference: one subtract + abs).
    - VectorE min-reduces each 128-wide segment: seg[p, s].  The DVE runs
      at 1 elem/cycle for reductions, which is the kernel's floor.
  Phase 2 -- exact argmin from segment minima (per query tile):
    - global min m = reduce_min(seg); first segment with seg == m via
      max_index (first-occurrence semantics match argmin's tie-break);
      concurrent indirect-DMA gathers of that segment's refs and accuracy
      rows; recompute ref - x (bit-identical) and search +-m with
      max_index; pick accuracy[w] via an iota==w one-hot dot product.

A per-chunk ScalarE "fence" (Copy of one bcast element) absorbs the
multi-queue DMA waits once per chunk, keeping per-instruction semaphore
waits cheap.  All comparisons are exact fp32, so the result matches the
jax reference bit-for-bit, including argmin tie-breaks.
"""
from contextlib import ExitStack

import numpy as np

import concourse.bass as bass
import concourse.bacc as bacc
import concourse.tile as tile
from concourse import mybir
from concourse._compat import with_exitstack
from concourse.bass_utils import run_bass_kernel_spmd

P = 128
N_CORES = 8
B = 4096
B_CORE = B // N_CORES  # 512
N = 65536
F = 4096               # refs per chunk (first chunk is split for fast start)
CHUNK_PLAN = [(0, 1024), (1024, 1024), (2048, 2048)] + [
    (off, F) for off in range(F, N, F)
]
N_QT = B_CORE // P     # 4 query tiles per core
W = 128                # segment width
S = F // W             # 16 segments per chunk
NSEG = N // W          # 512 segments total
MM = 512               # max moving free dim per matmul

FP32 = mybir.dt.float32
U32 = mybir.dt.uint32
I32 = mybir.dt.int32


@with_exitstack
def _nn_kernel(ctx: ExitStack, tc: tile.TileContext, xq, refs, ra, iota, out):
    nc = tc.nc

    bcast_pool = ctx.enter_context(tc.tile_pool(name="bcast", bufs=3))
    dist_pool = ctx.enter_context(tc.tile_pool(name="dist", bufs=4))
    small_pool = ctx.enter_context(tc.tile_pool(name="small", bufs=2))
    persist = ctx.enter_context(tc.tile_pool(name="persist", bufs=1))

    # Issue the first broadcast DMAs before anything else so ScalarE can
    # start as early as possible.
    early = []
    for off, flen in CHUNK_PLAN[:2]:
        bc = bcast_pool.tile([P, F], FP32, tag="bcast", name="bcast")
        nc.sync.dma_start(
            out=bc[:, :flen],
            in_=refs[off : off + flen][None, :].to_broadcast([P, flen]),
        )
        early.append(bc)

    x_sb = persist.tile([P, N_QT], FP32, tag="x_sb")
    nc.sync.dma_start(out=x_sb[:], in_=xq.rearrange("(q p) -> p q", p=P))
    neg_x = persist.tile([P, N_QT], FP32, tag="neg_x")
    nc.vector.tensor_scalar_mul(neg_x[:], x_sb[:], -1.0)

    # Per-qtile segment minima, filled chunk by chunk.
    segs = [
        persist.tile([P, NSEG], FP32, tag=f"seg{qt}", name=f"seg{qt}")
        for qt in range(N_QT)
    ]

    # ---- Phase 1: segment minima ----
    # Each chunk of refs is replicated to all 128 partitions by DMA;
    # ScalarE computes dist = |ref - x_p| (Abs activation with
    # per-partition bias), the DVE runs only the segment min-reduces.
    # A tiny per-chunk ACT "fence" (Copy of one bcast element) absorbs the
    # multi-queue DMA waits once, so the dist ops carry only their cheap
    # embedded WAR wait.
    fdummy = persist.tile([P, 1], FP32, tag="fdummy")
    iota_pw = persist.tile([P, W], FP32, tag="iota_pw")
    nc.sync.dma_start(out=iota_pw[:], in_=iota[None, :].to_broadcast([P, W]))
    stage = persist.tile([P, N_QT], FP32, tag="stage")

    def phase1(off, flen, qt, fence):
        dist = dist_pool.tile([P, F], FP32, tag="dist", name="dist")
        d_call = nc.scalar.activation(
            dist[:, :flen],
            bcast[:, :flen],
            mybir.ActivationFunctionType.Abs,
            bias=neg_x[:, qt : qt + 1],
            scale=1.0,
        )
        bass._add_dep_helper(
            d_call.ins, fence.ins, sync=False, reason="fence before dist"
        )
        nc.vector.tensor_reduce(
            segs[qt][:, off // W : (off + flen) // W],
            dist[:, :flen].rearrange("p (s w) -> p s w", w=W),
            axis=mybir.AxisListType.X,
            op=mybir.AluOpType.min,
        )

    # ---- Phase 2: exact argmin for one query tile ----
    def phase2(qt):
        gmin = small_pool.tile([P, 1], FP32, tag="gmin")
        nc.vector.tensor_reduce(
            gmin[:], segs[qt][:], axis=mybir.AxisListType.X, op=mybir.AluOpType.min
        )
        m8 = small_pool.tile([P, 8], FP32, tag="m8")
        nc.vector.tensor_copy(m8[:], gmin[:, 0:1].to_broadcast([P, 8]))
        s8 = small_pool.tile([P, 8], U32, tag="s8")
        nc.vector.max_index(s8[:], m8[:], segs[qt][:])
        # Gather the winning segment's refs+accuracy row (interleaved table,
        # one indirect DMA round-trip) for each lane.
        gra = small_pool.tile([P, 2 * W], FP32, tag="gra")
        nc.gpsimd.indirect_dma_start(
            out=gra[:],
            out_offset=None,
            in_=ra,
            in_offset=bass.IndirectOffsetOnAxis(ap=s8[:, 0:1], axis=0),
        )
        # Recompute ref - x for the gathered segment (bit-identical signed
        # diff) and search it for +gmin / -gmin; the smaller found index is
        # the first position with |diff| == gmin.
        dist_w = small_pool.tile([P, W], FP32, tag="dist_w")
        nc.vector.tensor_scalar(
            dist_w[:],
            gra[:, 0:W],
            x_sb[:, qt : qt + 1],
            None,
            op0=mybir.AluOpType.subtract,
        )
        mpm = small_pool.tile([P, 8], FP32, tag="mpm")
        nc.vector.tensor_copy(mpm[:, 0:4], gmin[:, 0:1].to_broadcast([P, 4]))
        nc.vector.tensor_scalar(
            mpm[:, 4:8],
            gmin[:, 0:1].to_broadcast([P, 4]),
            -1.0,
            None,
            op0=mybir.AluOpType.mult,
        )
        w8 = small_pool.tile([P, 8], U32, tag="w8")
        nc.vector.max_index(w8[:], mpm[:], dist_w[:])
        # Global index = seg * W + within-segment index (fp32 arithmetic is
        # exact for values < 2^24; a not-found slot becomes 2^32-1 in fp32
        # and loses the min).
        wp_f = small_pool.tile([P, 1], FP32, tag="wp_f")
        nc.vector.tensor_copy(wp_f[:], w8[:, 0:1])
        wm_f = small_pool.tile([P, 1], FP32, tag="wm_f")
        nc.vector.tensor_copy(wm_f[:], w8[:, 4:5])
        w_f = small_pool.tile([P, 1], FP32, tag="w_f")
        nc.vector.tensor_tensor(
            out=w_f[:], in0=wp_f[:], in1=wm_f[:], op=mybir.AluOpType.min
        )
        # accuracy[w]: one-hot select via iota == w, then a sum-reduce.
        sel = small_pool.tile([P, W], FP32, tag="sel")
        nc.vector.tensor_tensor(
            out=sel[:],
            in0=iota_pw[:],
            in1=w_f[:, 0:1].to_broadcast([P, W]),
            op=mybir.AluOpType.is_equal,
        )
        nc.vector.tensor_tensor(
            out=sel[:], in0=sel[:], in1=gra[:, W : 2 * W], op=mybir.AluOpType.mult
        )
        nc.vector.tensor_reduce(
            stage[:, qt : qt + 1],
            sel[:],
            axis=mybir.AxisListType.X,
            op=mybir.AluOpType.add,
        )

    for ci, (off, flen) in enumerate(CHUNK_PLAN):
        last = ci == len(CHUNK_PLAN) - 1
        if ci < 2:
            bcast = early[ci]
        else:
            bcast = bcast_pool.tile([P, F], FP32, tag="bcast", name="bcast")
            nc.sync.dma_start(
                out=bcast[:, :flen],
                in_=refs[off : off + flen][None, :].to_broadcast([P, flen]),
            )
        fence = nc.scalar.activation(
            fdummy[:], bcast[:, 0:1], mybir.ActivationFunctionType.Copy
        )
        for qt in range(N_QT):
            phase1(off, flen, qt, fence)
            if last:
                phase2(qt)
    nc.sync.dma_start(out=out.rearrange("(q p) -> p q", p=P), in_=stage[:])


_CACHED_NC = None


def _build():
    global _CACHED_NC
    if _CACHED_NC is not None:
        return _CACHED_NC
    nc = bacc.Bacc("TRN2", target_bir_lowering=False, debug=False)
    xq = nc.dram_tensor("xq", [B_CORE], FP32, kind="ExternalInput").ap()
    refs = nc.dram_tensor("refs", [N], FP32, kind="ExternalInput").ap()
    ra = nc.dram_tensor("ra", [NSEG, 2 * W], FP32, kind="ExternalInput").ap()
    iota = nc.dram_tensor("iota", [W], FP32, kind="ExternalInput").ap()
    out = nc.dram_tensor("out", [B_CORE], FP32, kind="ExternalOutput").ap()
    with tile.TileContext(nc) as tc:
        _nn_kernel(tc, xq, refs, ra, iota, out)
    nc.compile()
    _CACHED_NC = nc
    return nc


def kernel(x, input_tensor, accuracy_tensor):
    x = np.asarray(x, dtype=np.float32)
    refs = np.ascontiguousarray(np.asarray(input_tensor, dtype=np.float32))
    acc = np.ascontiguousarray(np.asarray(accuracy_tensor, dtype=np.float32))

    nc = _build()
    ra = np.ascontiguousarray(
        np.concatenate([refs.reshape(NSEG, W), acc.reshape(NSEG, W)], axis=1)
    ).astype(np.float32)
    iota = np.arange(W, dtype=np.float32)
    in_maps = [
        {
            "xq": np.ascontiguousarray(x[i * B_CORE : (i + 1) * B_CORE]),
            "refs": refs,
            "ra": ra,
            "iota": iota,
        }
        for i in range(N_CORES)
    ]
    res = run_bass_kernel_spmd(nc, in_maps, core_ids=list(range(N_CORES)))
    return np.concatenate([res.results[i]["out"] for i in range(N_CORES)])


# revision 33
# speedup vs baseline: 1.0117x; 1.0056x over previous
"""1-D nearest-neighbor retrieval kernel for Trainium2 (8 NeuronCores).

For each query x[b], finds argmin_n |input_tensor[n] - x[b]| and returns
accuracy_tensor[argmin].  Queries are sharded across the 8 cores (512 each,
4 query tiles of 128 partitions); the ref/accuracy tables are replicated.

Per-core pipeline (queries in SBUF partitions, refs in the free dim):
  Phase 1 -- segment minima (the O(B*N) bulk):
    - Each chunk of refs is partition-broadcast to [128, F] SBUF by DMA.
    - ScalarE computes dist = |ref - x_p| via activation(Abs, bias=-x_p)
      (bit-identical to the fp32 reference: one subtract + abs).
    - VectorE min-reduces each 128-wide segment: seg[p, s].  The DVE runs
      at 1 elem/cycle for reductions, which is the kernel's floor.
  Phase 2 -- exact argmin from segment minima (per query tile):
    - global min m = reduce_min(seg); first segment with seg == m via
      max_index (first-occurrence semantics match argmin's tie-break);
      one indirect-DMA gather of that segment's interleaved refs+accuracy
      row; recompute ref - x (bit-identical) and search +-m with
      max_index; pick accuracy[w] via an iota==w one-hot dot product.

A per-chunk ScalarE "fence" (Copy of one bcast element) absorbs the
multi-queue DMA waits once per chunk, keeping per-instruction semaphore
waits cheap.  All comparisons are exact fp32, so the result matches the
jax reference bit-for-bit, including argmin tie-breaks.
"""
from contextlib import ExitStack

import numpy as np

import concourse.bass as bass
import concourse.bacc as bacc
import concourse.tile as tile
from concourse import mybir
from concourse._compat import with_exitstack
from concourse.bass_utils import run_bass_kernel_spmd

P = 128
N_CORES = 8
B = 4096
B_CORE = B // N_CORES  # 512
N = 65536
F = 4096               # refs per chunk (first chunk is split for fast start)
CHUNK_PLAN = [(0, 1024), (1024, 1024), (2048, 2048)] + [
    (off, F) for off in range(F, N, F)
]
N_QT = B_CORE // P     # 4 query tiles per core
W = 128                # segment width
NSEG = N // W          # 512 segments total

FP32 = mybir.dt.float32
U32 = mybir.dt.uint32


@with_exitstack
def _nn_kernel(ctx: ExitStack, tc: tile.TileContext, xq, refs, ra, iota, out):
    nc = tc.nc

    bcast_pool = ctx.enter_context(tc.tile_pool(name="bcast", bufs=3))
    dist_pool = ctx.enter_context(tc.tile_pool(name="dist", bufs=4))
    small_pool = ctx.enter_context(tc.tile_pool(name="small", bufs=2))
    persist = ctx.enter_context(tc.tile_pool(name="persist", bufs=1))

    # Issue the first broadcast DMAs before anything else so ScalarE can
    # start as early as possible.
    early = []
    for off, flen in CHUNK_PLAN[:2]:
        bc = bcast_pool.tile([P, F], FP32, tag="bcast", name="bcast")
        nc.sync.dma_start(
            out=bc[:, :flen],
            in_=refs[off : off + flen][None, :].to_broadcast([P, flen]),
        )
        early.append(bc)

    x_sb = persist.tile([P, N_QT], FP32, tag="x_sb")
    nc.sync.dma_start(out=x_sb[:], in_=xq.rearrange("(q p) -> p q", p=P))
    neg_x = persist.tile([P, N_QT], FP32, tag="neg_x")
    nc.vector.tensor_scalar_mul(neg_x[:], x_sb[:], -1.0)

    # Per-qtile segment minima, filled chunk by chunk.
    segs = [
        persist.tile([P, NSEG], FP32, tag=f"seg{qt}", name=f"seg{qt}")
        for qt in range(N_QT)
    ]

    # ---- Phase 1: segment minima ----
    # Each chunk of refs is replicated to all 128 partitions by DMA;
    # ScalarE computes dist = |ref - x_p| (Abs activation with
    # per-partition bias), the DVE runs only the segment min-reduces.
    # A tiny per-chunk ACT "fence" (Copy of one bcast element) absorbs the
    # multi-queue DMA waits once, so the dist ops carry only their cheap
    # embedded WAR wait.
    fdummy = persist.tile([P, 1], FP32, tag="fdummy")
    iota_pw = persist.tile([P, W], FP32, tag="iota_pw")
    nc.sync.dma_start(out=iota_pw[:], in_=iota[None, :].to_broadcast([P, W]))
    stage = persist.tile([P, N_QT], FP32, tag="stage")

    def phase1(off, flen, qt, fence):
        dist = dist_pool.tile([P, F], FP32, tag="dist", name="dist")
        d_call = nc.scalar.activation(
            dist[:, :flen],
            bcast[:, :flen],
            mybir.ActivationFunctionType.Abs,
            bias=neg_x[:, qt : qt + 1],
            scale=1.0,
        )
        bass._add_dep_helper(
            d_call.ins, fence.ins, sync=False, reason="fence before dist"
        )
        nc.vector.tensor_reduce(
            segs[qt][:, off // W : (off + flen) // W],
            dist[:, :flen].rearrange("p (s w) -> p s w", w=W),
            axis=mybir.AxisListType.X,
            op=mybir.AluOpType.min,
        )

    # ---- Phase 2: exact argmin for one query tile ----
    def phase2(qt):
        gmin = small_pool.tile([P, 1], FP32, tag="gmin")
        nc.vector.tensor_reduce(
            gmin[:], segs[qt][:], axis=mybir.AxisListType.X, op=mybir.AluOpType.min
        )
        m8 = small_pool.tile([P, 8], FP32, tag="m8")
        nc.vector.tensor_copy(m8[:], gmin[:, 0:1].to_broadcast([P, 8]))
        s8 = small_pool.tile([P, 8], U32, tag="s8")
        nc.vector.max_index(s8[:], m8[:], segs[qt][:])
        # Gather the winning segment's refs+accuracy row (interleaved table,
        # one indirect DMA round-trip) for each lane.
        gra = small_pool.tile([P, 2 * W], FP32, tag="gra")
        nc.gpsimd.indirect_dma_start(
            out=gra[:],
            out_offset=None,
            in_=ra,
            in_offset=bass.IndirectOffsetOnAxis(ap=s8[:, 0:1], axis=0),
        )
        # Recompute ref - x for the gathered segment (bit-identical signed
        # diff) and search it for +gmin / -gmin; the smaller found index is
        # the first position with |diff| == gmin.
        dist_w = small_pool.tile([P, W], FP32, tag="dist_w")
        nc.vector.tensor_scalar(
            dist_w[:],
            gra[:, 0:W],
            x_sb[:, qt : qt + 1],
            None,
            op0=mybir.AluOpType.subtract,
        )
        mpm = small_pool.tile([P, 8], FP32, tag="mpm")
        nc.vector.tensor_copy(mpm[:, 0:4], gmin[:, 0:1].to_broadcast([P, 4]))
        nc.vector.tensor_scalar(
            mpm[:, 4:8],
            gmin[:, 0:1].to_broadcast([P, 4]),
            -1.0,
            None,
            op0=mybir.AluOpType.mult,
        )
        w8 = small_pool.tile([P, 8], U32, tag="w8")
        nc.vector.max_index(w8[:], mpm[:], dist_w[:])
        # Within-segment winner = min of the two found positions (a
        # not-found slot becomes 2^32-1 in fp32 and loses the min).
        wp_f = small_pool.tile([P, 1], FP32, tag="wp_f")
        nc.vector.tensor_copy(wp_f[:], w8[:, 0:1])
        wm_f = small_pool.tile([P, 1], FP32, tag="wm_f")
        nc.vector.tensor_copy(wm_f[:], w8[:, 4:5])
        w_f = small_pool.tile([P, 1], FP32, tag="w_f")
        nc.vector.tensor_tensor(
            out=w_f[:], in0=wp_f[:], in1=wm_f[:], op=mybir.AluOpType.min
        )
        # accuracy[w]: one-hot select via iota == w, then a sum-reduce.
        sel = small_pool.tile([P, W], FP32, tag="sel")
        nc.vector.tensor_tensor(
            out=sel[:],
            in0=iota_pw[:],
            in1=w_f[:, 0:1].to_broadcast([P, W]),
            op=mybir.AluOpType.is_equal,
        )
        nc.vector.tensor_tensor(
            out=sel[:], in0=sel[:], in1=gra[:, W : 2 * W], op=mybir.AluOpType.mult
        )
        nc.vector.tensor_reduce(
            stage[:, qt : qt + 1],
            sel[:],
            axis=mybir.AxisListType.X,
            op=mybir.AluOpType.add,
        )

    for ci, (off, flen) in enumerate(CHUNK_PLAN):
        last = ci == len(CHUNK_PLAN) - 1
        if ci < 2:
            bcast = early[ci]
        else:
            bcast = bcast_pool.tile([P, F], FP32, tag="bcast", name="bcast")
            nc.sync.dma_start(
                out=bcast[:, :flen],
                in_=refs[off : off + flen][None, :].to_broadcast([P, flen]),
            )
        fence = nc.scalar.activation(
            fdummy[:], bcast[:, 0:1], mybir.ActivationFunctionType.Copy
        )
        for qt in range(N_QT):
            phase1(off, flen, qt, fence)
            if last:
                phase2(qt)
    nc.sync.dma_start(out=out.rearrange("(q p) -> p q", p=P), in_=stage[:])


_CACHED_NC = None


def _build():
    global _CACHED_NC
    if _CACHED_NC is not None:
        return _CACHED_NC
    nc = bacc.Bacc("TRN2", target_bir_lowering=False, debug=False)
    xq = nc.dram_tensor("xq", [B_CORE], FP32, kind="ExternalInput").ap()
    refs = nc.dram_tensor("refs", [N], FP32, kind="ExternalInput").ap()
    ra = nc.dram_tensor("ra", [NSEG, 2 * W], FP32, kind="ExternalInput").ap()
    iota = nc.dram_tensor("iota", [W], FP32, kind="ExternalInput").ap()
    out = nc.dram_tensor("out", [B_CORE], FP32, kind="ExternalOutput").ap()
    with tile.TileContext(nc) as tc:
        _nn_kernel(tc, xq, refs, ra, iota, out)
    nc.compile()
    _CACHED_NC = nc
    return nc


def kernel(x, input_tensor, accuracy_tensor):
    x = np.asarray(x, dtype=np.float32)
    refs = np.ascontiguousarray(np.asarray(input_tensor, dtype=np.float32))
    acc = np.ascontiguousarray(np.asarray(accuracy_tensor, dtype=np.float32))

    nc = _build()
    ra = np.ascontiguousarray(
        np.concatenate([refs.reshape(NSEG, W), acc.reshape(NSEG, W)], axis=1)
    ).astype(np.float32)
    iota = np.arange(W, dtype=np.float32)
    in_maps = [
        {
            "xq": np.ascontiguousarray(x[i * B_CORE : (i + 1) * B_CORE]),
            "refs": refs,
            "ra": ra,
            "iota": iota,
        }
        for i in range(N_CORES)
    ]
    res = run_bass_kernel_spmd(nc, in_maps, core_ids=list(range(N_CORES)))
    return np.concatenate([res.results[i]["out"] for i in range(N_CORES)])
